# revision 2
# baseline (speedup 1.0000x reference)
"""TRN2 Bass kernel: causal-conv QKV projections + query-axis-softmax attention.

Problem (per batch element b):
    q = causal_conv1d(x, Wq) + bq        # [T, U], K=3 taps, left-pad 2
    k = causal_conv1d(x, Wk) + bk
    v = causal_conv1d(x, Wv) + bv
    s[q_, k_] = (q[q_] . k[k_]) / sqrt(U)
    P = softmax(s, axis=q_)              # normalized over the QUERY axis
    out[q_, d] = sum_k P[q_, k_] * v[k_, d]

Sharding: data-parallel over batch. B == 8 == n_cores, one batch element per
NeuronCore, same program on all cores (SPMD), different inputs.

Per-core algorithm:
  1. Load x [2048, 512], transpose on the PE (64x 128x128 transposes) into
     XT [cin, t] with 2 zero columns of left-padding for the causal taps.
  2. QT[u, t], KT[u, t] via 12 accumulating fp32r matmuls per PSUM fill
     (3 taps x 4 cin chunks), lhsT = W[j][cin_chunk, u_chunk], rhs = shifted
     XT slice.  Bias added on the ScalarE during the PSUM->SBUF drain.
  3. V[t, u] (natural layout) similarly, lhsT = shifted XT slice, rhs = W tile;
     bias via an extra ones-trick matmul into the same accumulation group.
     V stored fp16.
  4. S^T[k, q] tiles [128, 2048]: 4 fp32r matmuls per 512-wide q chunk
     (contract u). exp() on ScalarE (scale=1/sqrt(U)) with accum_out giving
     the per-k row sum Z; ET stored fp16. No max subtraction: |s| <~ 6 so
     exp(s) is far from fp32/fp16 range limits.
  5. V[k] *= 1/Z[k] (per-partition scalar on the DVE).
  6. out[q, d] = sum over 16 k-tiles: ET[kt][:, q_chunk].T @ V[kt], fp16
     matmuls accumulated in PSUM, drained fp32 and DMA'd out.

All matmul moving operands are N=512 so fp32r runs at 1 cycle/row.
"""

import os
import sys

sys.path.insert(0, "/opt/trn_rl_repo")

import numpy as np

T = 2048
C = 512  # input channels
U = 512  # units
KW = 3  # conv taps (causal, left-pad KW-1)
P = 128
NCH = C // P  # 4 cin chunks
NUC = U // P  # 4 u chunks
NTT = T // P  # 16 t (and k) tiles
NTC = T // 512  # 4 t 512-col chunks
SCALE = 1.0 / float(np.sqrt(U))
NCORES = 8
# debug aid: 1 = stop after QKV (dump v), 2 = stop after exp (dump et), 3 = full
_PHASE = int(os.environ.get("KPHASE", "3"))
# timing aid: repeat the whole kernel body KREP times inside one NEFF so the
# per-rep device time can be extracted from wall-clock differences (the axon
# RPC overhead per dispatch is ~14ms, dwarfing a single ~300us kernel).
_NREP = int(os.environ.get("KREP", "1"))
# KF16=1: x/W/QT/KT in fp16 instead of f32r. fp16 streams at 1 cycle/row with
# fast-weight-load + LDW pipelining on the PE; fp32r matmuls self-load their
# weights (no FWL, serialized load) and measured ~2.2x slower than modeled.
_F16 = os.environ.get("KF16", "1") == "1"

_CACHE = {}


def _build(nrep=None, f16=None, phase=None):
    nrep = _NREP if nrep is None else nrep
    f16 = _F16 if f16 is None else f16
    phase = _PHASE if phase is None else phase
    key = ("nc", nrep, f16, phase)
    if key in _CACHE:
        return _CACHE[key]

    import concourse.bass as bass  # noqa: F401
    import concourse.mybir as mybir
    import concourse.tile as tile
    from concourse import bacc, masks

    f32 = mybir.dt.float32
    f32r = mybir.dt.float32r
    f16dt = mybir.dt.float16
    fpe = f16dt if f16 else f32r  # dtype of PE matmul operands
    AF = mybir.ActivationFunctionType
    AX = mybir.AxisListType

    nc = bacc.Bacc("TRN2", target_bir_lowering=False, debug=False, num_devices=NCORES)

    x_d = nc.dram_tensor("x", [T, C], f32, kind="ExternalInput").ap()
    wq_d = nc.dram_tensor("wq", [KW, C, U], f32, kind="ExternalInput").ap()
    wk_d = nc.dram_tensor("wk", [KW, C, U], f32, kind="ExternalInput").ap()
    wv_d = nc.dram_tensor("wv", [KW, C, U], f32, kind="ExternalInput").ap()
    bq_d = nc.dram_tensor("bq", [U], f32, kind="ExternalInput").ap()
    bk_d = nc.dram_tensor("bk", [U], f32, kind="ExternalInput").ap()
    bv_d = nc.dram_tensor("bv", [U], f32, kind="ExternalInput").ap()
    out_d = nc.dram_tensor("out", [T, U], f32, kind="ExternalOutput").ap()

    with tile.TileContext(nc) as tc:
        with (
            tc.tile_pool(name="const", bufs=1) as constp,
            tc.tile_pool(name="qkt", bufs=1) as qktp,
            tc.tile_pool(name="vpool", bufs=1) as vpool,
            tc.tile_pool(name="zpool", bufs=2) as zpool,
            tc.tile_pool(name="ostage", bufs=4) as outp,
            tc.tile_pool(name="acc", bufs=2, space="PSUM") as accp,
        ):
            # ---------------- constants ----------------
            ident = constp.tile([P, P], f32, name="ident")
            masks.make_identity(nc, ident[:])
            # Memset can't write f32r (invalid ISA); bounce through f32
            # scratch tiles and TensorCopy (which can round to f32r).
            zsc = constp.tile([P, U], f32, name="zsc")
            nc.vector.memset(zsc[:], 0.0)
            osc = constp.tile([P, U], f32, name="osc")
            nc.vector.memset(osc[:], 1.0)
            ones128 = constp.tile([P, P], fpe, name="ones128")
            nc.vector.tensor_copy(ones128[:], osc[:, 0:P])
            bvpad = constp.tile([P, U], fpe, name="bvpad")
            nc.vector.tensor_copy(bvpad[:], zsc[:])
            if f16:
                nc.gpsimd.dma_start(
                    bvpad[0:1, :], bv_d[:].rearrange("(o u) -> o u", o=1)
                )
            else:
                nc.sync.dma_start(
                    bvpad[0:1, :],
                    bv_d[:].rearrange("(o u) -> o u", o=1).bitcast(f32r),
                )

            bq_t = []
            bk_t = []
            for uc in range(NUC):
                bqc = constp.tile([P, 1], f32, name=f"bq{uc}")
                nc.sync.dma_start(bqc[:, 0], bq_d[uc * P : (uc + 1) * P])
                bq_t.append(bqc)
                bkc = constp.tile([P, 1], f32, name=f"bk{uc}")
                nc.sync.dma_start(bkc[:, 0], bk_d[uc * P : (uc + 1) * P])
                bk_t.append(bkc)

            for _rep in range(nrep):
                # persistent SBUF arrays
                qt = [
                    qktp.tile([P, T], fpe, name=f"qt{d}", tag=f"qt{d}") for d in range(NUC)
                ]
                kt = [
                    qktp.tile([P, T], fpe, name=f"kt{d}", tag=f"kt{d}") for d in range(NUC)
                ]
                vt = [
                    vpool.tile([P, U], f16dt, name=f"v{i}", tag=f"v{i}") for i in range(NTT)
                ]

                # ---------------- phase 1: x load + transpose + QKV ----------------
                with (
                    tc.tile_pool(name="xstage", bufs=4) as xp,
                    tc.tile_pool(name="xtp", bufs=1) as xtp,
                    tc.tile_pool(name="wp", bufs=2) as wp,
                ):
                    xt = [
                        xtp.tile([P, 2 + T], fpe, name=f"xt{c}", tag=f"xt{c}")
                        for c in range(NCH)
                    ]
                    for c in range(NCH):
                        nc.vector.tensor_copy(xt[c][:, 0:2], zsc[:, 0:2])

                    def load_w(dram, jname):
                        # HWDGE DMA (fast) + DVE copy-cast; SWDGE cast-DMA is
                        # far slower and stalls the PE on the weight loads.
                        tiles = []
                        for j in range(KW):
                            row = []
                            for c in range(NCH):
                                wt = wp.tile(
                                    [P, U], fpe, name=f"w{jname}{j}_{c}", tag=f"w{j}_{c}"
                                )
                                if f16:
                                    wstg = wp.tile(
                                        [P, U], f32, name=f"wstg{jname}{j}_{c}",
                                        tag="wstg", bufs=4,
                                    )
                                    nc.sync.dma_start(
                                        wstg[:], dram[j, c * P : (c + 1) * P, :]
                                    )
                                    nc.vector.tensor_copy(wt[:], wstg[:])
                                else:
                                    nc.sync.dma_start(
                                        wt[:],
                                        dram[j, c * P : (c + 1) * P, :].bitcast(f32r),
                                    )
                                row.append(wt)
                            tiles.append(row)
                        return tiles

                    # Wv first: the V fills interleave with the x transposes
                    # below (they only need the already-transposed t-tiles), so
                    # the PE has matmul work while x tiles stream in from HBM.
                    wv_t = load_w(wv_d, "v")

                    jc = [(j, c) for j in range(KW) for c in range(NCH)]
                    for g in range(NTT // 4):
                        # transpose 4 t-tiles into xt
                        for i in range(4):
                            ti = g * 4 + i
                            xs = xp.tile([P, C], f32, name="xs", tag="xs")
                            nc.sync.dma_start(xs[:], x_d[ti * P : (ti + 1) * P, :])
                            acc = accp.tile([P, 4, 512], f32, name="acc", tag="acc")
                            for c in range(NCH):
                                nc.tensor.transpose(
                                    acc[:, c, 0:P], xs[:, c * P : (c + 1) * P], ident[:]
                                )
                                nc.vector.tensor_copy(
                                    xt[c][:, 2 + ti * P : 2 + (ti + 1) * P],
                                    acc[:, c, 0:P],
                                )
                        # V fill for this group: out [t_tile 128, u 512], fp16
                        acc = accp.tile([P, 4, 512], f32, name="acc", tag="acc")
                        for i in range(4):
                            ti = g * 4 + i
                            for idx, (j, c) in enumerate(jc):
                                lhsT = xt[c][:, ti * P + j : ti * P + j + P]
                                nc.tensor.matmul(
                                    acc[:, i, :],
                                    lhsT,
                                    wv_t[j][c][:],
                                    start=(idx == 0),
                                    stop=False,
                                )
                            # bias: ones.T @ [bv; 0...] adds bv to every row
                            nc.tensor.matmul(
                                acc[:, i, :],
                                ones128[:],
                                bvpad[:],
                                start=False,
                                stop=True,
                            )
                        for i in range(4):
                            nc.vector.tensor_copy(vt[g * 4 + i][:], acc[:, i, :])

                    wq_t = load_w(wq_d, "q")
                    wk_t = load_w(wk_d, "k")

                    # QT / KT fills: out [u_chunk 128, t], 12 matmuls per fill
                    def qk_fill(w_tiles, dst, bias_tiles):
                        for uc in range(NUC):
                            acc = accp.tile([P, 4, 512], f32, name="acc", tag="acc")
                            for idx, (j, c) in enumerate(
                                [(j, c) for j in range(KW) for c in range(NCH)]
                            ):
                                lhsT = w_tiles[j][c][:, uc * P : (uc + 1) * P]
                                for tch in range(NTC):
                                    rhs = xt[c][:, tch * 512 + j : tch * 512 + j + 512]
                                    nc.tensor.matmul(
                                        acc[:, tch, :],
                                        lhsT,
                                        rhs,
                                        start=(idx == 0),
                                        stop=(idx == KW * NCH - 1),
                                    )
                            for tch in range(NTC):
                                nc.scalar.activation(
                                    dst[uc][:, tch * 512 : (tch + 1) * 512],
                                    acc[:, tch, :],
                                    AF.Identity,
                                    bias=bias_tiles[uc][:, 0:1],
                                    scale=1.0,
                                )

                    qk_fill(wq_t, qt, bq_t)
                    qk_fill(wk_t, kt, bk_t)

                if phase == 1:
                    for i in range(NTT):
                        ost = outp.tile([P, U], f32, name="ost", tag="ost")
                        nc.vector.tensor_copy(ost[:], vt[i][:])
                        nc.sync.dma_start(out_d[i * P : (i + 1) * P, :], ost[:])

                # ---------------- phase 2: S^T tiles, exp, Z, V scaling ------------
                with tc.tile_pool(name="etp", bufs=1) as etp:
                    et = [
                        etp.tile([P, T], f16dt, name=f"et{k}", tag=f"et{k}")
                        for k in range(NTT)
                    ]
                    for ktile in range(NTT if phase >= 2 else 0):
                        acc = accp.tile([P, 4, 512], f32, name="acc", tag="acc")
                        for d in range(NUC):
                            lhsT = kt[d][:, ktile * P : (ktile + 1) * P]
                            for qch in range(NTC):
                                nc.tensor.matmul(
                                    acc[:, qch, :],
                                    lhsT,
                                    qt[d][:, qch * 512 : (qch + 1) * 512],
                                    start=(d == 0),
                                    stop=(d == NUC - 1),
                                )
                        zp = zpool.tile([P, 4], f32, name="zp", tag="zp")
                        for qch in range(NTC):
                            nc.scalar.activation(
                                et[ktile][:, qch * 512 : (qch + 1) * 512],
                                acc[:, qch, :],
                                AF.Exp,
                                scale=SCALE,
                                accum_out=zp[:, qch : qch + 1],
                            )
                        zs = zpool.tile([P, 1], f32, name="zs", tag="zs")
                        nc.vector.reduce_sum(zs[:, 0:1], zp[:], axis=AX.X)
                        zr = zpool.tile([P, 1], f32, name="zr", tag="zr")
                        nc.vector.reciprocal(zr[:, 0:1], zs[:, 0:1])
                        nc.vector.tensor_scalar_mul(vt[ktile][:], vt[ktile][:], zr[:, 0:1])

                    if phase == 2:
                        for i in range(NTT):
                            ost = outp.tile([P, U], f32, name="ost", tag="ost")
                            nc.vector.tensor_copy(ost[:], et[i][:, 0:U])
                            nc.sync.dma_start(out_d[i * P : (i + 1) * P, :], ost[:])

                    # ------------- phase 3: context matmuls + output ---------------
                    for g in range(NTT // 4 if phase >= 3 else 0):
                        acc = accp.tile([P, 4, 512], f32, name="acc", tag="acc")
                        for ktile in range(NTT):
                            for i in range(4):
                                qtile = g * 4 + i
                                nc.tensor.matmul(
                                    acc[:, i, :],
                                    et[ktile][:, qtile * P : (qtile + 1) * P],
                                    vt[ktile][:],
                                    start=(ktile == 0),
                                    stop=(ktile == NTT - 1),
                                )
                        for i in range(4):
                            qtile = g * 4 + i
                            ost = outp.tile([P, U], f32, name="ost", tag="ost")
                            nc.vector.tensor_copy(ost[:], acc[:, i, :])
                            nc.sync.dma_start(out_d[qtile * P : (qtile + 1) * P, :], ost[:])

    nc.compile()

    # The libneuronxla NEFF cache keys on the HLO module, which does NOT
    # include the Bass BIR embedded in the custom call's backend_config --
    # two different Bass programs with identical I/O signatures collide and
    # silently reuse each other's NEFF. Bust it with a dummy input whose
    # shape is derived from the program content hash.
    import hashlib

    h = int.from_bytes(
        hashlib.sha256(mybir.module_to_json_bytes(nc.m)).digest()[:8], "big"
    )
    d0 = (h % 509) + 1
    d1 = ((h // 509) % 509) + 1
    nc.dram_tensor("cachebust", [1, d0, d1], f32, kind="ExternalInput")
    _CACHE[key + ("cachebust",)] = (1, d0, d1)
    nc._cachebust_shape = (1, d0, d1)

    _CACHE[key] = nc
    return nc


def _shared_inputs(inputs, nc):
    """Host-side preprocessing of the per-core-identical inputs."""
    shared = {
        "wq": np.ascontiguousarray(np.asarray(inputs["Wq"], dtype=np.float32)),
        "wk": np.ascontiguousarray(np.asarray(inputs["Wk"], dtype=np.float32)),
        "wv": np.ascontiguousarray(np.asarray(inputs["Wv"], dtype=np.float32)),
        "bq": np.ascontiguousarray(np.asarray(inputs["bq"], dtype=np.float32)),
        "bk": np.ascontiguousarray(np.asarray(inputs["bk"], dtype=np.float32)),
        "bv": np.ascontiguousarray(np.asarray(inputs["bv"], dtype=np.float32)),
    }
    shared["cachebust"] = np.zeros(nc._cachebust_shape, dtype=np.float32)
    return shared


def _run(inputs, trace=False):
    """Run on all 8 cores. Returns (stacked output [8, T, U], BassKernelResults)."""
    from concourse.bass_utils import run_bass_kernel_spmd

    nc = _build()
    x = np.ascontiguousarray(np.asarray(inputs["x"], dtype=np.float32))
    assert x.shape == (NCORES, T, C), x.shape
    shared = _shared_inputs(inputs, nc)
    in_maps = [{"x": x[b], **shared} for b in range(NCORES)]
    res = run_bass_kernel_spmd(
        nc, in_maps, core_ids=list(range(NCORES)), trace=trace
    )
    out = np.stack([res.results[b]["out"] for b in range(NCORES)], axis=0)
    return out, res


def kernel(**inputs) -> np.ndarray:
    out, _ = _run(inputs, trace=False)
    return out

